# revision 16
# baseline (speedup 1.0000x reference)
"""Multi-head attention (B=16, L=S=1024, D=P=512, H=8) on 8 TRN2 NeuronCores.

Strategy: pure data parallelism over the batch -- each core computes the full
attention block for 2 batch elements.  Activations arrive pre-transposed and
fp8-DoubleRow-interleaved so every projection contracts 256 rows per matmul:

  per batch element b (all on one core):
    QT[P,L] = Wq.T @ qT + 16*bq   fp8e4 DoubleRow (2 MMs instead of 4),
    KT[P,S] = Wk.T @ kT + 16*bk   weights carry a x16 host prescale so fp8e4
                                  stays in normal range; the x256 on scores
                                  is folded into the exp scale
    V [S,P] = vT.T @ Wv + bv      bf16 (fp8 here is not softmax-protected);
                                  fp16 in 128-wide head blocks [1.0, 0*63,
                                  V_h] so the A@V matmul emits softmax sums
                                  at psum partition 0 and the head output at
                                  partitions 64..127
    per head pair (even h on PE row-tile T0, odd h on T8, concurrent):
      expT[S,L] = exp(scale/256 * K_h @ Q_h^T)  ACT table exp + DVE
                  Schraudolph split; score matmuls for the two heads
                  interleave instruction-by-instruction so the 64x128
                  row-tiles run pairwise (~1.7x)
      OT_h[E,L] = V128_h.T @ expT  full-array; rows normalized by the DVE
                  fast-recip + GpSimd partition broadcast + DVE multiply
    out[L,D] = OT.T-contraction with Wo + bo (f32r), emitted per L-half

Scheduling: heads run in even/odd pairs; the previous pair's A@V block sits
between a pair's score sp-groups 0-1 and 2-3 (2 tiling-mode switches per
pair) so the PE keeps full-array work while ACT/DVE drain score psums.
"""

import numpy as np

B, L, S, D, P, H, E = 16, 1024, 1024, 512, 512, 8, 64
NCORES = 8
BPC = B // NCORES  # batch elements per core
SCALE = 1.0 / float(np.sqrt(E))
WSCALE = 16.0  # host premultiplier on Wq/Wk/Wv so fp8e4 stays in normal range
QKSCALE = SCALE / (WSCALE * WSCALE)  # scores carry x256 from the two prescales

# Schraudolph exp, fp16 flavor: the DVE computes v = A*score + B in fp32 and
# converts to int16; the bit pattern read back as fp16 is ~exp(qkscale*score)
# with ~3% sawtooth error (softmax normalization cancels any uniform scale
# error, only the sawtooth shape survives).
SCHRAUD_A = float(2**10 / np.log(2)) * QKSCALE
SCHRAUD_B = float((15 - 0.043677448) * 2**10)  # 15 = fp16 exponent bias
# which of the 8 S-chunks per (h,lc) go to the DVE
DVE_STS_EVEN = frozenset((5, 6, 7))
DVE_STS_ODD = frozenset((5, 6, 7))


def _exp_plan(sp, dve_sts):
    """Per score-psum pair sp (sts 2sp, 2sp+1): list of (j0, j1, on_dve) ops."""
    a = (sp * 2) in dve_sts
    b = (sp * 2 + 1) in dve_sts
    if a == b:
        return [(0, 2, a)]
    return [(0, 1, a), (1, 2, b)]

_CACHE = {}
LAST_RESULTS = None  # stashed BassKernelResults for test harness introspection


def _build():
    """Build (once) the Bass program executed identically on all 8 cores."""
    if "nc" in _CACHE:
        return _CACHE["nc"]

    from contextlib import ExitStack

    import concourse.bass as bass
    import concourse.mybir as mybir
    import concourse.tile as tile
    from concourse import bacc

    f32 = mybir.dt.float32
    f32r = mybir.dt.float32r
    f16 = mybir.dt.float16
    i16 = mybir.dt.int16
    bf16 = mybir.dt.bfloat16
    fp8 = mybir.dt.float8e4
    AF = mybir.ActivationFunctionType
    ALU = mybir.AluOpType
    DR = mybir.MatmulPerfMode.DoubleRow

    nc = bacc.Bacc("TRN2", target_bir_lowering=False, debug=False)

    # DR-interleaved activations: element (b, g, p, k, l) = xT[g*256+k*128+p, l]
    qT = nc.dram_tensor("qT", [BPC, 2, 128, 2, L], fp8, kind="ExternalInput").ap()
    kT = nc.dram_tensor("kT", [BPC, 2, 128, 2, S], fp8, kind="ExternalInput").ap()
    # V path stays bf16: fp8 here is NOT softmax-protected -- its ~8% element
    # error lands directly in the output (measured 1.3e-2 rel err vs 4.5e-3)
    vT = nc.dram_tensor("vT", [BPC, D, S], bf16, kind="ExternalInput").ap()
    Wq = nc.dram_tensor("Wq", [2, 128, 2, P], fp8, kind="ExternalInput").ap()
    Wk = nc.dram_tensor("Wk", [2, 128, 2, P], fp8, kind="ExternalInput").ap()
    Wv = nc.dram_tensor("Wv", [D, P], bf16, kind="ExternalInput").ap()
    Wo = nc.dram_tensor("Wo", [P, D], f32, kind="ExternalInput").ap()
    bq_col = nc.dram_tensor("bq_col", [128, 4], f32, kind="ExternalInput").ap()
    bk_col = nc.dram_tensor("bk_col", [128, 4], f32, kind="ExternalInput").ap()
    bv_row = nc.dram_tensor("bv_row", [P], f32, kind="ExternalInput").ap()
    bo_row = nc.dram_tensor("bo_row", [D], f32, kind="ExternalInput").ap()
    out = nc.dram_tensor("out", [BPC, L, D], f32, kind="ExternalOutput").ap()

    def bcast_ap(src, n=128):
        # [N] DRAM vector (or [1, N] SBUF row) -> [n, N] partition-broadcast AP
        return bass.AP(tensor=src.tensor, offset=src.offset, ap=[[0, n]] + src.ap[-1:])

    with tile.TileContext(nc) as tc, ExitStack() as ctx:
        consts = ctx.enter_context(tc.tile_pool(name="consts", bufs=1))
        xT_pool = ctx.enter_context(tc.tile_pool(name="xT", bufs=2))
        acts = ctx.enter_context(tc.tile_pool(name="acts", bufs=2))
        exp_pool = ctx.enter_context(tc.tile_pool(name="exp", bufs=6))
        small = ctx.enter_context(tc.tile_pool(name="small", bufs=2))
        out_pool = ctx.enter_context(tc.tile_pool(name="outp", bufs=3))
        psum = ctx.enter_context(tc.tile_pool(name="psum", bufs=3, space="PSUM"))
        psum_ot = ctx.enter_context(tc.tile_pool(name="psum_ot", bufs=2, space="PSUM"))

        # ---- constants: DR weights [128, 2, N] (contraction pairs packed on
        # partitions), per-g tiles so the first matmul waits only its own DMA.
        Wq_sb = [consts.tile([128, 2, P], fp8, tag=f"Wq{g}", name=f"Wq{g}") for g in range(2)]
        Wk_sb = [consts.tile([128, 2, P], fp8, tag=f"Wk{g}", name=f"Wk{g}") for g in range(2)]
        Wv_sb = [consts.tile([128, P], bf16, tag=f"Wv{dt}", name=f"Wv{dt}") for dt in range(4)]
        Wo_sb = [consts.tile([128, D], f32r, tag=f"Wo{dt}", name=f"Wo{dt}") for dt in range(4)]
        bq_sb = consts.tile([128, 4], f32, tag="bq")
        bk_sb = consts.tile([128, 4], f32, tag="bk")
        bv_sb = consts.tile([128, P], f32, tag="bv")
        bo_sb = consts.tile([128, D], f32, tag="bo")
        # V in 128-wide head blocks: col h*128 = 1.0, cols +1..63 = 0, cols
        # +64..127 = head h of V.  The OT matmul's [128,128] stationary then
        # emits the softmax denominator at psum PARTITION 0 (ones column) and
        # the head output at partitions 64..127 -- both PSUM-aligned offsets,
        # so the fast-recip custom-DVE op reads the sums straight from PSUM
        # (nonzero psum partition offsets trip a HW bug in custom-DVE ops).
        V_sb = consts.tile([128, 8, 8 * 128], f16, tag="V")  # [S-part, st, 1024]
        Vv = V_sb.rearrange("p s (h e) -> p s h e", e=128)
        nc.vector.memset(Vv[:, :, :, 1:64], 0.0)
        nc.vector.memset(Vv[:, :, :, 0:1], 1.0)

        # PE warm-up: dummy matmuls over a small GpSimd-memset tile while the
        # first weight/activation DMAs are still in flight.  The tensor engine
        # needs ~3us of continuous execution to ramp to 2.4GHz.
        warm_sb = consts.tile([128, 512], f16, tag="warm")
        nc.gpsimd.memset(warm_sb, 1.0)
        warm_ps = psum.tile([128, 1024], f32, tag="scores", name="warm_ps")
        for _ in range(8):
            nc.tensor.matmul(
                warm_ps[0:64, 0:512],
                warm_sb[:, 0:64],
                warm_sb,
                start=True,
                stop=True,
            )

        def load_xT(src, b, name):
            ts = [xT_pool.tile([128, 2, L], fp8, tag=f"{name}{g}", name=f"{name}{g}")
                  for g in range(2)]
            for g in range(2):
                nc.sync.dma_start(out=ts[g], in_=src[b, g])
            return ts

        def load_vT(b):
            ts = [xT_pool.tile([128, S], bf16, tag=f"vT_sb{dt}", name=f"vT_sb{dt}")
                  for dt in range(4)]
            view = vT[b].rearrange("(t p) l -> p t l", p=128)
            for dt in range(4):
                nc.sync.dma_start(out=ts[dt], in_=view[:, dt, :])
            return ts

        def load_w(W_sb, Wsrc):
            for g in range(2):
                nc.sync.dma_start(out=W_sb[g], in_=Wsrc[g])

        # The first psum group consumes (Wq[g], qT[g]) in g order: issue the
        # DMAs in exactly that order, alternating across queues.
        qT0_sb = [xT_pool.tile([128, 2, L], fp8, tag=f"qT_sb{g}", name=f"qT0_{g}")
                  for g in range(2)]
        nc.scalar.dma_start(out=Wq_sb[0], in_=Wq[0])
        nc.gpsimd.dma_start(out=qT0_sb[0], in_=qT[0, 0])
        nc.sync.dma_start(out=Wq_sb[1], in_=Wq[1])
        nc.gpsimd.dma_start(out=qT0_sb[1], in_=qT[0, 1])
        first = {"qT_sb": qT0_sb}
        nc.sync.dma_start(out=bq_sb, in_=bq_col)
        load_w(Wk_sb, Wk)
        nc.sync.dma_start(out=bk_sb, in_=bk_col)
        first["kT_sb"] = load_xT(kT, 0, "kT_sb")
        Wv_view = Wv.rearrange("(t p) n -> p t n", p=128)
        for dt in range(4):
            nc.sync.dma_start(out=Wv_sb[dt], in_=Wv_view[:, dt, :])
        nc.gpsimd.dma_start(out=bv_sb, in_=bcast_ap(bv_row))
        first["vT_sb"] = load_vT(0)
        Wo_view = Wo.rearrange("(t p) n -> p t n", p=128).bitcast(f32r)
        for dt in range(4):
            nc.sync.dma_start(out=Wo_sb[dt], in_=Wo_view[:, dt, :])
        nc.gpsimd.dma_start(out=bo_sb, in_=bcast_ap(bo_row))

        # Filler-job queues, shared ACROSS batches so one batch's out-proj
        # tail weaves into the next batch's pair stream.  urgent jobs are
        # A@V retire quarters (must clear within their pair); spare jobs are
        # V-proj / out-proj tiles that can lag.
        urgent_q, spare_q = [], []

        for b in range(BPC):
            if b == 0:
                qT_sb, kT_sb, vT_sb = first["qT_sb"], first["kT_sb"], first["vT_sb"]
            else:
                qT_sb = load_xT(qT, b, "qT_sb")
                kT_sb = load_xT(kT, b, "kT_sb")
                vT_sb = load_vT(b)

            QT_sb = acts.tile([128, 4, L], f16, tag="QT")  # [P-part, ptile, L]
            KT_sb = acts.tile([128, 4, S], f16, tag="KT")
            OT_sb = acts.tile([128, 4, L], f32r, tag="OT")  # [P-part, ptile, L]

            # ---- QT / KT projections, fp8 DoubleRow (K=256 per matmul):
            # psum[p, l] = 16 * sum_d W[d, p] * xT[d, l]; bias+copy on the ACT
            # (Identity with per-partition bias AP, bias x16 on host).
            for W_sb, b_sb, X_sb, Y_sb in (
                (Wq_sb, bq_sb, qT_sb, QT_sb),
                (Wk_sb, bk_sb, kT_sb, KT_sb),
            ):
                for pt in range(4):
                    for lc in range(2):
                        ps = psum.tile([128, 1024], f32, tag="scores", name="ps")[:, 0:512]
                        for g in range(2):
                            nc.tensor.matmul(
                                ps,
                                W_sb[g][:, :, pt * 128:(pt + 1) * 128],
                                X_sb[g][:, :, lc * 512:(lc + 1) * 512],
                                start=(g == 0),
                                stop=(g == 1),
                                perf_mode=DR,
                            )
                        nc.scalar.activation(
                            out=Y_sb[:, pt, lc * 512:(lc + 1) * 512],
                            in_=ps,
                            func=AF.Identity,
                            bias=b_sb[:, pt:pt + 1],
                        )

            # ---- attention: heads run in even/odd PAIRS whose score matmuls
            # interleave the two 64x128 PE row-tiles (bass derives
            # tile_position from the operands' base partitions) and run
            # concurrently.
            def emit_norm(h, lc, ps_o):
                pt_h, po_h = h // 2, (h % 2) * 64
                lsl = slice(lc * 512, (lc + 1) * 512)
                recip_sb = small.tile([1, 512], f32, tag="recip", name="recip_sb")
                nc.vector.reciprocal_approx_fast(out=recip_sb, in_=ps_o[0:1, :])
                rep_sb = small.tile([64, 512], f32, tag="rep", name="rep_sb")
                nc.gpsimd.partition_broadcast(rep_sb, recip_sb, channels=64)
                nc.vector.tensor_mul(
                    OT_sb[po_h:po_h + 64, pt_h, lsl], ps_o[64:128, :], rep_sb
                )

            def av_mm(ps_av, pexp, ph, st):
                nc.tensor.matmul(
                    ps_av,
                    V_sb[:, st, ph * 128:(ph + 1) * 128],
                    pexp[:, st, :],
                    start=(st == 0),
                    stop=(st == 7),
                )

            class AvRetire:
                """A@V for a retired pair, emitted as 4 quarter-jobs of 4
                full-array matmuls (sts 2q, 2q+1 for both heads); norms are
                issued with the final quarter."""

                def __init__(self, pr):
                    self.pr = pr
                    self.q = 0
                    self.ps = None

                def quarter(self):
                    ppt, plc, pexp_e, pexp_o = self.pr
                    if self.ps is None:
                        self.ps = (
                            psum_ot.tile([128, 512], f32, tag="ot", name="ps_ave"),
                            psum_ot.tile([128, 512], f32, tag="ot", name="ps_avo"),
                        )
                    for st in (self.q * 2, self.q * 2 + 1):
                        av_mm(self.ps[0], pexp_e, 2 * ppt, st)
                        av_mm(self.ps[1], pexp_o, 2 * ppt + 1, st)
                    self.q += 1
                    if self.q == 4:
                        emit_norm(2 * ppt, plc, self.ps[0])
                        emit_norm(2 * ppt + 1, plc, self.ps[1])

            def push_retire(pr):
                r = AvRetire(pr)
                urgent_q.extend([r.quarter] * 4)

            def emit_pair(pt, lc):
                # Paired scores: one sp-group (4 matmuls interleaving the two
                # 64x128 PE row-tiles) then one or two filler jobs (psum-free
                # full-array work) so exp always has fresh psums to drain AND
                # the PE never idles on the scores pool rotation.
                expT_e = exp_pool.tile([128, 8, 512], f16, tag="expT", name="expT_e")
                expT_o = exp_pool.tile([128, 8, 512], f16, tag="expT", name="expT_o")
                lsl = slice(lc * 512, (lc + 1) * 512)

                def scores_sp(sp):
                    ps_e = psum.tile([128, 1024], f32, tag="scores", name="ps_e")
                    ps_o = psum.tile([128, 1024], f32, tag="scores", name="ps_o")
                    for j in range(2):
                        st = sp * 2 + j
                        ssl = slice(st * 128, (st + 1) * 128)
                        nc.tensor.matmul(
                            ps_e[:, j * 512:(j + 1) * 512],
                            KT_sb[0:64, pt, ssl],
                            QT_sb[0:64, pt, lsl],
                            start=True,
                            stop=True,
                        )
                        nc.tensor.matmul(
                            ps_o[:, j * 512:(j + 1) * 512],
                            KT_sb[64:128, pt, ssl],
                            QT_sb[64:128, pt, lsl],
                            start=True,
                            stop=True,
                        )
                    # exp split: most chunks on ACT (table exp); DVE takes the
                    # tail chunks as a Schraudolph bit-trick (int16 bits <-
                    # scores*(A*qkscale)+B, read back as fp16; GPSIMD can't
                    # help here, it has no PSUM access on HW).
                    for ps_x, expT_x, dve_sts in (
                        (ps_e, expT_e, DVE_STS_EVEN),
                        (ps_o, expT_o, DVE_STS_ODD),
                    ):
                        psv = ps_x.rearrange("p (a b) -> p a b", b=512)
                        for j0, j1, on_dve in _exp_plan(sp, dve_sts):
                            if on_dve:
                                nc.vector.tensor_scalar(
                                    out=expT_x[:, sp * 2 + j0:sp * 2 + j1, :].bitcast(i16),
                                    in0=psv[:, j0:j1, :],
                                    scalar1=SCHRAUD_A,
                                    scalar2=SCHRAUD_B,
                                    op0=ALU.mult,
                                    op1=ALU.add,
                                )
                            else:
                                nc.scalar.activation(
                                    out=expT_x[:, sp * 2 + j0:sp * 2 + j1, :],
                                    in_=psv[:, j0:j1, :],
                                    func=AF.Exp,
                                    scale=QKSCALE,
                                )

                for sp in range(4):
                    scores_sp(sp)
                    # drain fillers: one urgent (A@V retire quarter), plus one
                    # spare when there's backlog pressure or nothing urgent
                    ran_urgent = False
                    if urgent_q:
                        urgent_q.pop(0)()
                        ran_urgent = True
                    if spare_q and (
                        not ran_urgent or len(urgent_q) + len(spare_q) >= 7
                    ):
                        spare_q.pop(0)()
                return expT_e, expT_o

            def out_lt_job(lt, bb=b):
                # out projection rows lt*128..lt*128+127 (needs all heads of
                # that L-half in OT_sb): psum[l, d] = sum_p OT[p,l]*Wo[p,d]
                OT_cap = OT_sb

                def f():
                    ps = psum.tile([128, 1024], f32, tag="scores", name="ps")[:, 0:512]
                    for pt in range(4):
                        nc.tensor.matmul(
                            ps,
                            OT_cap[:, pt, lt * 128:(lt + 1) * 128],
                            Wo_sb[pt],
                            start=(pt == 0),
                            stop=(pt == 3),
                        )
                    o_sb = out_pool.tile([128, 512], f32, tag="osb")
                    nc.vector.tensor_add(o_sb, ps, bo_sb)
                    nc.sync.dma_start(out=out[bb, lt * 128:(lt + 1) * 128, :], in_=o_sb)

                return f

            def vproj_job(st0, vT_cap, V_cap):
                # V projection (bf16) for sts st0, st0+1:
                # psum[s, p] = sum_d vT[d, s] * Wv[d, p]
                def f():
                    for st in (st0, st0 + 1):
                        ps = psum_ot.tile([128, 512], f32, tag="ot", name="ps")
                        for dt in range(4):
                            nc.tensor.matmul(
                                ps,
                                vT_cap[dt][:, st * 128:(st + 1) * 128],
                                Wv_sb[dt],
                                start=(dt == 0),
                                stop=(dt == 3),
                            )
                        nc.vector.tensor_add(
                            V_cap[:, st, :, 64:128],
                            ps.rearrange("p (h e) -> p h e", e=64),
                            bv_sb.rearrange("p (h e) -> p h e", e=64),
                        )

                return f

            # lc-outer pair order: (3,0) retires two pairs before the end, so
            # out-proj half 0 runs mid-stream; only half 1 remains in the
            # tail.  The last pair weaves two pairs' retires (backlog-drain).
            seq = [(pt, lc) for lc in (0, 1) for pt in range(4)]
            pending = []
            for k, (pt, lc) in enumerate(seq):
                if k >= 2:
                    push_retire(pending.pop(0))
                if k == 0:
                    spare_q.extend([vproj_job(2 * i, vT_sb, Vv) for i in range(4)])
                pending.append((pt, lc, *emit_pair(pt, lc)))
                if (pt, lc) == (1, 1):
                    # (3,0) was retired during this pair: half 0 is ready
                    spare_q.extend([out_lt_job(lt) for lt in range(4)])
            # tail: flush leftovers, retire the last two pairs, then half 1
            # (carried into the next batch's pair stream when one follows).
            while spare_q:
                spare_q.pop(0)()
            for pr in pending:
                push_retire(pr)
                while urgent_q:
                    urgent_q.pop(0)()
            if b == BPC - 1:
                for lt in range(4, 8):
                    out_lt_job(lt)()
            else:
                spare_q.extend([out_lt_job(lt) for lt in range(4, 8)])

    nc.compile()
    _CACHE["nc"] = nc
    return nc


def _in_maps(inputs):
    import ml_dtypes

    e4 = ml_dtypes.float8_e4m3fn
    f = lambda a: np.ascontiguousarray(np.asarray(a, dtype=np.float32))
    queries, keys, values = f(inputs["queries"]), f(inputs["keys"]), f(inputs["values"])
    Wq, Wk, Wv, Wo = f(inputs["Wq"]), f(inputs["Wk"]), f(inputs["Wv"]), f(inputs["Wo"])
    bq, bk, bv, bo = f(inputs["bq"]), f(inputs["bk"]), f(inputs["bv"]), f(inputs["bo"])

    def w_dr(W):
        # [D, N] -> [g, p, k, N] fp8 with d = g*256 + k*128 + p, x16 prescale
        return np.ascontiguousarray(
            (W * WSCALE).reshape(2, 2, 128, -1).transpose(0, 2, 1, 3).astype(e4)
        )

    def x_dr(x):
        # [b, L, D] -> [b, g, p, k, L] fp8 with d = g*256 + k*128 + p
        xt = x.transpose(0, 2, 1)  # [b, D, L]
        bsz = xt.shape[0]
        return np.ascontiguousarray(
            xt.reshape(bsz, 2, 2, 128, -1).transpose(0, 1, 3, 2, 4).astype(e4)
        )

    b16 = ml_dtypes.bfloat16
    shared = {
        "Wq": w_dr(Wq), "Wk": w_dr(Wk), "Wv": np.ascontiguousarray(Wv.astype(b16)),
        "Wo": Wo,
        "bq_col": np.ascontiguousarray(bq.reshape(4, 128).T) * WSCALE,
        "bk_col": np.ascontiguousarray(bk.reshape(4, 128).T) * WSCALE,
        "bv_row": bv, "bo_row": bo,
    }
    maps = []
    for c in range(NCORES):
        sl = slice(BPC * c, BPC * (c + 1))
        maps.append({
            "qT": x_dr(queries[sl]),
            "kT": x_dr(keys[sl]),
            "vT": np.ascontiguousarray(values[sl].transpose(0, 2, 1).astype(b16)),
            **shared,
        })
    return maps


def kernel(**inputs) -> np.ndarray:
    global LAST_RESULTS
    from concourse import bass_utils

    nc = _build()
    maps = _in_maps(inputs)
    res = bass_utils.run_bass_kernel_spmd(nc, maps, core_ids=list(range(NCORES)))
    LAST_RESULTS = res
    return np.concatenate([res.results[c]["out"] for c in range(NCORES)], axis=0)


# revision 22
# speedup vs baseline: 1.0173x; 1.0173x over previous
"""Multi-head attention (B=16, L=S=1024, D=P=512, H=8) on 8 TRN2 NeuronCores.

Strategy: pure data parallelism over the batch -- each core computes the full
attention block for 2 batch elements.  Activations arrive pre-transposed and
fp8-DoubleRow-interleaved so every projection contracts 256 rows per matmul:

  per batch element b (all on one core):
    QT[P,L] = Wq.T @ qT + 16*bq   fp8e4 DoubleRow (2 MMs instead of 4),
    KT[P,S] = Wk.T @ kT + 16*bk   weights carry a x16 host prescale so fp8e4
                                  stays in normal range; the x256 on scores
                                  is folded into the exp scale
    V [S,P] = vT.T @ Wv + bv      bf16 (fp8 here is not softmax-protected);
                                  fp16 in 128-wide head blocks [1.0, 0*63,
                                  V_h] so the A@V matmul emits softmax sums
                                  at psum partition 0 and the head output at
                                  partitions 64..127
    per head pair (even h on PE row-tile T0, odd h on T8, concurrent):
      expT[S,L] = exp(scale/256 * K_h @ Q_h^T)  ACT table exp + DVE
                  Schraudolph split; score matmuls for the two heads
                  interleave instruction-by-instruction so the 64x128
                  row-tiles run pairwise (~1.7x)
      OT_h[E,L] = V128_h.T @ expT  full-array; rows normalized by the DVE
                  fast-recip + GpSimd partition broadcast + DVE multiply
    out[L,D] = OT.T-contraction with Wo + bo (f32r), emitted per L-half

Scheduling: heads run in even/odd pairs; the previous pair's A@V block sits
between a pair's score sp-groups 0-1 and 2-3 (2 tiling-mode switches per
pair) so the PE keeps full-array work while ACT/DVE drain score psums.
"""

import numpy as np

B, L, S, D, P, H, E = 16, 1024, 1024, 512, 512, 8, 64
NCORES = 8
BPC = B // NCORES  # batch elements per core
SCALE = 1.0 / float(np.sqrt(E))
WSCALE = 16.0  # host premultiplier on Wq/Wk/Wv so fp8e4 stays in normal range
QKSCALE = SCALE / (WSCALE * WSCALE)  # scores carry x256 from the two prescales

# Schraudolph exp, fp16 flavor: the DVE computes v = A*score + B in fp32 and
# converts to int16; the bit pattern read back as fp16 is ~exp(qkscale*score)
# with ~3% sawtooth error (softmax normalization cancels any uniform scale
# error, only the sawtooth shape survives).
SCHRAUD_A = float(2**10 / np.log(2)) * QKSCALE
SCHRAUD_B = float((15 - 0.043677448) * 2**10)  # 15 = fp16 exponent bias
# which of the 8 S-chunks per (h,lc) go to the DVE
DVE_STS_EVEN = frozenset((5, 6, 7))
DVE_STS_ODD = frozenset((5, 6, 7))


def _exp_plan(sp, dve_sts):
    """Per score-psum pair sp (sts 2sp, 2sp+1): list of (j0, j1, on_dve) ops."""
    a = (sp * 2) in dve_sts
    b = (sp * 2 + 1) in dve_sts
    if a == b:
        return [(0, 2, a)]
    return [(0, 1, a), (1, 2, b)]

_CACHE = {}
LAST_RESULTS = None  # stashed BassKernelResults for test harness introspection


def _build():
    """Build (once) the Bass program executed identically on all 8 cores."""
    if "nc" in _CACHE:
        return _CACHE["nc"]

    from contextlib import ExitStack

    import concourse.bass as bass
    import concourse.mybir as mybir
    import concourse.tile as tile
    from concourse import bacc

    f32 = mybir.dt.float32
    f32r = mybir.dt.float32r
    f16 = mybir.dt.float16
    i16 = mybir.dt.int16
    bf16 = mybir.dt.bfloat16
    fp8 = mybir.dt.float8e4
    AF = mybir.ActivationFunctionType
    ALU = mybir.AluOpType
    DR = mybir.MatmulPerfMode.DoubleRow

    nc = bacc.Bacc("TRN2", target_bir_lowering=False, debug=False)

    # DR-interleaved activations: element (b, g, p, k, l) = xT[g*256+k*128+p, l]
    qT = nc.dram_tensor("qT", [BPC, 2, 128, 2, L], fp8, kind="ExternalInput").ap()
    kT = nc.dram_tensor("kT", [BPC, 2, 128, 2, S], fp8, kind="ExternalInput").ap()
    # V path stays bf16: fp8 here is NOT softmax-protected -- its ~8% element
    # error lands directly in the output (measured 1.3e-2 rel err vs 4.5e-3)
    vT = nc.dram_tensor("vT", [BPC, D, S], bf16, kind="ExternalInput").ap()
    Wq = nc.dram_tensor("Wq", [2, 128, 2, P], fp8, kind="ExternalInput").ap()
    Wk = nc.dram_tensor("Wk", [2, 128, 2, P], fp8, kind="ExternalInput").ap()
    Wv = nc.dram_tensor("Wv", [D, P], bf16, kind="ExternalInput").ap()
    Wo = nc.dram_tensor("Wo", [P, D], f32, kind="ExternalInput").ap()
    bq_col = nc.dram_tensor("bq_col", [128, 4], f32, kind="ExternalInput").ap()
    bk_col = nc.dram_tensor("bk_col", [128, 4], f32, kind="ExternalInput").ap()
    bv_row = nc.dram_tensor("bv_row", [P], f32, kind="ExternalInput").ap()
    bo_row = nc.dram_tensor("bo_row", [D], f32, kind="ExternalInput").ap()
    out = nc.dram_tensor("out", [BPC, L, D], f32, kind="ExternalOutput").ap()

    def bcast_ap(src, n=128):
        # [N] DRAM vector (or [1, N] SBUF row) -> [n, N] partition-broadcast AP
        return bass.AP(tensor=src.tensor, offset=src.offset, ap=[[0, n]] + src.ap[-1:])

    with tile.TileContext(nc) as tc, ExitStack() as ctx:
        consts = ctx.enter_context(tc.tile_pool(name="consts", bufs=1))
        xT_pool = ctx.enter_context(tc.tile_pool(name="xT", bufs=2))
        acts = ctx.enter_context(tc.tile_pool(name="acts", bufs=2))
        exp_pool = ctx.enter_context(tc.tile_pool(name="exp", bufs=6))
        small = ctx.enter_context(tc.tile_pool(name="small", bufs=2))
        out_pool = ctx.enter_context(tc.tile_pool(name="outp", bufs=3))
        psum = ctx.enter_context(tc.tile_pool(name="psum", bufs=3, space="PSUM"))
        psum_ot = ctx.enter_context(tc.tile_pool(name="psum_ot", bufs=2, space="PSUM"))

        # ---- constants: DR weights [128, 2, N] (contraction pairs packed on
        # partitions), per-g tiles so the first matmul waits only its own DMA.
        Wq_sb = [consts.tile([128, 2, P], fp8, tag=f"Wq{g}", name=f"Wq{g}") for g in range(2)]
        Wk_sb = [consts.tile([128, 2, P], fp8, tag=f"Wk{g}", name=f"Wk{g}") for g in range(2)]
        Wv_sb = [consts.tile([128, P], bf16, tag=f"Wv{dt}", name=f"Wv{dt}") for dt in range(4)]
        Wo_sb = [consts.tile([128, D], f32r, tag=f"Wo{dt}", name=f"Wo{dt}") for dt in range(4)]
        bq_sb = consts.tile([128, 4], f32, tag="bq")
        bk_sb = consts.tile([128, 4], f32, tag="bk")
        bv_sb = consts.tile([128, P], f32, tag="bv")
        bo_sb = consts.tile([128, D], f32, tag="bo")
        # V in 128-wide head blocks: col h*128 = 1.0, cols +1..63 = 0, cols
        # +64..127 = head h of V.  The OT matmul's [128,128] stationary then
        # emits the softmax denominator at psum PARTITION 0 (ones column) and
        # the head output at partitions 64..127 -- both PSUM-aligned offsets,
        # so the fast-recip custom-DVE op reads the sums straight from PSUM
        # (nonzero psum partition offsets trip a HW bug in custom-DVE ops).
        V_sb = consts.tile([128, 8, 8 * 128], f16, tag="V")  # [S-part, st, 1024]
        Vv = V_sb.rearrange("p s (h e) -> p s h e", e=128)
        nc.vector.memset(Vv[:, :, :, 1:64], 0.0)
        nc.vector.memset(Vv[:, :, :, 0:1], 1.0)

        # PE warm-up: dummy matmuls over a small GpSimd-memset tile while the
        # first weight/activation DMAs are still in flight.  The tensor engine
        # needs ~3us of continuous execution to ramp to 2.4GHz.
        warm_sb = consts.tile([128, 512], f16, tag="warm")
        nc.gpsimd.memset(warm_sb, 1.0)
        warm_ps = psum.tile([128, 1024], f32, tag="scores", name="warm_ps")
        for _ in range(8):
            nc.tensor.matmul(
                warm_ps[0:64, 0:512],
                warm_sb[:, 0:64],
                warm_sb,
                start=True,
                stop=True,
            )
        # engine warm-ups in the DMA dead zone: the FIRST Identity/Exp on the
        # ACT triggers an activation-table load and the first
        # PartitionBroadcast on GPSIMD pays a ~6us IRAM library load; issue
        # tiny dummies now so those one-time costs don't stall the pipeline.
        wf32 = consts.tile([1, 16], f32, tag="wf32")
        wfid = consts.tile([1, 16], f16, tag="wfid")
        wfex = consts.tile([1, 16], f16, tag="wfex")
        wfsc = consts.tile([1, 16], f16, tag="wfsc")
        wfrc = consts.tile([1, 16], f32, tag="wfrc")
        wfbc = consts.tile([16, 16], f32, tag="wfbc")
        nc.vector.memset(wf32, 1.0)
        nc.scalar.activation(out=wfid, in_=wf32, func=AF.Identity,
                             bias=wf32[:, 0:1])
        nc.scalar.activation(out=wfex, in_=wf32, func=AF.Exp, scale=QKSCALE)
        nc.vector.tensor_scalar(out=wfsc.bitcast(i16), in0=wf32,
                                scalar1=SCHRAUD_A, scalar2=SCHRAUD_B,
                                op0=ALU.mult, op1=ALU.add)
        nc.vector.reciprocal_approx_fast(out=wfrc, in_=wf32)
        nc.gpsimd.partition_broadcast(wfbc, wfrc, channels=16)
        def load_xT(src, b, name):
            ts = [xT_pool.tile([128, 2, L], fp8, tag=f"{name}{g}", name=f"{name}{g}")
                  for g in range(2)]
            for g in range(2):
                nc.sync.dma_start(out=ts[g], in_=src[b, g])
            return ts

        def load_vT(b):
            ts = [xT_pool.tile([128, S], bf16, tag=f"vT_sb{dt}", name=f"vT_sb{dt}")
                  for dt in range(4)]
            view = vT[b].rearrange("(t p) l -> p t l", p=128)
            for dt in range(4):
                nc.sync.dma_start(out=ts[dt], in_=view[:, dt, :])
            return ts

        def load_w(W_sb, Wsrc):
            for g in range(2):
                nc.sync.dma_start(out=W_sb[g], in_=Wsrc[g])

        # The first psum group consumes (Wq[g], qT[g]) in g order: issue the
        # DMAs in exactly that order, alternating across queues.
        qT0_sb = [xT_pool.tile([128, 2, L], fp8, tag=f"qT_sb{g}", name=f"qT0_{g}")
                  for g in range(2)]
        nc.scalar.dma_start(out=Wq_sb[0], in_=Wq[0])
        nc.gpsimd.dma_start(out=qT0_sb[0], in_=qT[0, 0])
        nc.sync.dma_start(out=Wq_sb[1], in_=Wq[1])
        nc.gpsimd.dma_start(out=qT0_sb[1], in_=qT[0, 1])
        first = {"qT_sb": qT0_sb}
        nc.sync.dma_start(out=bq_sb, in_=bq_col)
        load_w(Wk_sb, Wk)
        nc.sync.dma_start(out=bk_sb, in_=bk_col)
        first["kT_sb"] = load_xT(kT, 0, "kT_sb")
        Wv_view = Wv.rearrange("(t p) n -> p t n", p=128)
        for dt in range(4):
            nc.sync.dma_start(out=Wv_sb[dt], in_=Wv_view[:, dt, :])
        nc.gpsimd.dma_start(out=bv_sb, in_=bcast_ap(bv_row))
        first["vT_sb"] = load_vT(0)
        Wo_view = Wo.rearrange("(t p) n -> p t n", p=128).bitcast(f32r)
        for dt in range(4):
            nc.sync.dma_start(out=Wo_sb[dt], in_=Wo_view[:, dt, :])
        nc.gpsimd.dma_start(out=bo_sb, in_=bcast_ap(bo_row))

        # Filler-job queues, shared ACROSS batches so one batch's out-proj
        # tail weaves into the next batch's pair stream.  urgent jobs are
        # A@V retire quarters (must clear within their pair); spare jobs are
        # V-proj / out-proj tiles that can lag.
        urgent_q, spare_q = [], []

        for b in range(BPC):
            if b == 0:
                qT_sb, kT_sb, vT_sb = first["qT_sb"], first["kT_sb"], first["vT_sb"]
            else:
                qT_sb = load_xT(qT, b, "qT_sb")
                kT_sb = load_xT(kT, b, "kT_sb")
                vT_sb = load_vT(b)

            QT_sb = acts.tile([128, 4, L], f16, tag="QT")  # [P-part, ptile, L]
            KT_sb = acts.tile([128, 4, S], f16, tag="KT")
            OT_sb = acts.tile([128, 4, L], f32r, tag="OT")  # [P-part, ptile, L]

            # ---- QT / KT projections, fp8 DoubleRow (K=256 per matmul):
            # psum[p, l] = 16 * sum_d W[d, p] * xT[d, l]; bias+copy on the ACT
            # (Identity with per-partition bias AP, bias x16 on host).
            for W_sb, b_sb, X_sb, Y_sb in (
                (Wq_sb, bq_sb, qT_sb, QT_sb),
                (Wk_sb, bk_sb, kT_sb, KT_sb),
            ):
                for pt in range(4):
                    for lc in range(2):
                        ps = psum.tile([128, 1024], f32, tag="scores", name="ps")[:, 0:512]
                        for g in range(2):
                            nc.tensor.matmul(
                                ps,
                                W_sb[g][:, :, pt * 128:(pt + 1) * 128],
                                X_sb[g][:, :, lc * 512:(lc + 1) * 512],
                                start=(g == 0),
                                stop=(g == 1),
                                perf_mode=DR,
                            )
                        nc.scalar.activation(
                            out=Y_sb[:, pt, lc * 512:(lc + 1) * 512],
                            in_=ps,
                            func=AF.Identity,
                            bias=b_sb[:, pt:pt + 1],
                        )

            # ---- attention: heads run in even/odd PAIRS whose score matmuls
            # interleave the two 64x128 PE row-tiles (bass derives
            # tile_position from the operands' base partitions) and run
            # concurrently.
            def emit_norm(h, lc, ps_o):
                pt_h, po_h = h // 2, (h % 2) * 64
                lsl = slice(lc * 512, (lc + 1) * 512)
                recip_sb = small.tile([1, 512], f32, tag="recip", name="recip_sb")
                nc.vector.reciprocal_approx_fast(out=recip_sb, in_=ps_o[0:1, :])
                rep_sb = small.tile([64, 512], f32, tag="rep", name="rep_sb")
                nc.gpsimd.partition_broadcast(rep_sb, recip_sb, channels=64)
                nc.vector.tensor_mul(
                    OT_sb[po_h:po_h + 64, pt_h, lsl], ps_o[64:128, :], rep_sb
                )

            def av_mm(ps_av, pexp, ph, st):
                nc.tensor.matmul(
                    ps_av,
                    V_sb[:, st, ph * 128:(ph + 1) * 128],
                    pexp[:, st, :],
                    start=(st == 0),
                    stop=(st == 7),
                )

            class AvRetire:
                """A@V for a retired pair, emitted as 4 quarter-jobs of 4
                full-array matmuls (sts 2q, 2q+1 for both heads); norms are
                issued with the final quarter."""

                def __init__(self, pr):
                    self.pr = pr
                    self.q = 0
                    self.ps = None

                def quarter(self):
                    ppt, plc, pexp_e, pexp_o = self.pr
                    if self.ps is None:
                        self.ps = (
                            psum_ot.tile([128, 512], f32, tag="ot", name="ps_ave"),
                            psum_ot.tile([128, 512], f32, tag="ot", name="ps_avo"),
                        )
                    for st in (self.q * 2, self.q * 2 + 1):
                        av_mm(self.ps[0], pexp_e, 2 * ppt, st)
                        av_mm(self.ps[1], pexp_o, 2 * ppt + 1, st)
                    self.q += 1
                    if self.q == 4:
                        emit_norm(2 * ppt, plc, self.ps[0])
                        emit_norm(2 * ppt + 1, plc, self.ps[1])

            def push_retire(pr):
                r = AvRetire(pr)
                urgent_q.extend([r.quarter] * 4)

            def emit_pair(pt, lc):
                # Paired scores: one sp-group (4 matmuls interleaving the two
                # 64x128 PE row-tiles) then one or two filler jobs (psum-free
                # full-array work) so exp always has fresh psums to drain AND
                # the PE never idles on the scores pool rotation.
                expT_e = exp_pool.tile([128, 8, 512], f16, tag="expT", name="expT_e")
                expT_o = exp_pool.tile([128, 8, 512], f16, tag="expT", name="expT_o")
                lsl = slice(lc * 512, (lc + 1) * 512)

                def scores_sp(sp):
                    ps_e = psum.tile([128, 1024], f32, tag="scores", name="ps_e")
                    ps_o = psum.tile([128, 1024], f32, tag="scores", name="ps_o")
                    for j in range(2):
                        st = sp * 2 + j
                        ssl = slice(st * 128, (st + 1) * 128)
                        nc.tensor.matmul(
                            ps_e[:, j * 512:(j + 1) * 512],
                            KT_sb[0:64, pt, ssl],
                            QT_sb[0:64, pt, lsl],
                            start=True,
                            stop=True,
                        )
                        nc.tensor.matmul(
                            ps_o[:, j * 512:(j + 1) * 512],
                            KT_sb[64:128, pt, ssl],
                            QT_sb[64:128, pt, lsl],
                            start=True,
                            stop=True,
                        )
                    # exp split: most chunks on ACT (table exp); DVE takes the
                    # tail chunks as a Schraudolph bit-trick (int16 bits <-
                    # scores*(A*qkscale)+B, read back as fp16; GPSIMD can't
                    # help here, it has no PSUM access on HW).
                    for ps_x, expT_x, dve_sts in (
                        (ps_e, expT_e, DVE_STS_EVEN),
                        (ps_o, expT_o, DVE_STS_ODD),
                    ):
                        psv = ps_x.rearrange("p (a b) -> p a b", b=512)
                        for j0, j1, on_dve in _exp_plan(sp, dve_sts):
                            if on_dve:
                                nc.vector.tensor_scalar(
                                    out=expT_x[:, sp * 2 + j0:sp * 2 + j1, :].bitcast(i16),
                                    in0=psv[:, j0:j1, :],
                                    scalar1=SCHRAUD_A,
                                    scalar2=SCHRAUD_B,
                                    op0=ALU.mult,
                                    op1=ALU.add,
                                )
                            else:
                                nc.scalar.activation(
                                    out=expT_x[:, sp * 2 + j0:sp * 2 + j1, :],
                                    in_=psv[:, j0:j1, :],
                                    func=AF.Exp,
                                    scale=QKSCALE,
                                )

                for sp in range(4):
                    scores_sp(sp)
                    # drain fillers: one urgent (A@V retire quarter), plus one
                    # spare when there's backlog pressure or nothing urgent
                    ran_urgent = False
                    if urgent_q:
                        urgent_q.pop(0)()
                        ran_urgent = True
                    if spare_q and (
                        not ran_urgent or len(urgent_q) + len(spare_q) >= 6
                    ):
                        spare_q.pop(0)()
                return expT_e, expT_o

            def out_lt_job(lt, bb=b):
                # out projection rows lt*128..lt*128+127 (needs all heads of
                # that L-half in OT_sb): psum[l, d] = sum_p OT[p,l]*Wo[p,d]
                OT_cap = OT_sb

                def f():
                    ps = psum.tile([128, 1024], f32, tag="scores", name="ps")[:, 0:512]
                    for pt in range(4):
                        nc.tensor.matmul(
                            ps,
                            OT_cap[:, pt, lt * 128:(lt + 1) * 128],
                            Wo_sb[pt],
                            start=(pt == 0),
                            stop=(pt == 3),
                        )
                    o_sb = out_pool.tile([128, 512], f32, tag="osb")
                    nc.vector.tensor_add(o_sb, ps, bo_sb)
                    nc.sync.dma_start(out=out[bb, lt * 128:(lt + 1) * 128, :], in_=o_sb)

                return f

            def vproj_job(st, vT_cap, V_cap):
                # V projection (bf16) for one st: psum[s,p] = sum_d vT[d,s]Wv[d,p]
                def f():
                    ps = psum_ot.tile([128, 512], f32, tag="ot", name="ps")
                    for dt in range(4):
                        nc.tensor.matmul(
                            ps,
                            vT_cap[dt][:, st * 128:(st + 1) * 128],
                            Wv_sb[dt],
                            start=(dt == 0),
                            stop=(dt == 3),
                        )
                    nc.vector.tensor_add(
                        V_cap[:, st, :, 64:128],
                        ps.rearrange("p (h e) -> p h e", e=64),
                        bv_sb.rearrange("p (h e) -> p h e", e=64),
                    )

                return f

            # lc-outer pair order: (3,0) retires two pairs before the end, so
            # out-proj half 0 runs mid-stream; only half 1 remains in the
            # tail.  The last pair weaves two pairs' retires (backlog-drain).
            seq = [(pt, lc) for lc in (0, 1) for pt in range(4)]
            pending = []
            for k, (pt, lc) in enumerate(seq):
                if k >= 2:
                    push_retire(pending.pop(0))
                if k == 0:
                    # urgent so all 8 st jobs drain in P0/P1, strictly before
                    # the first A@V retire (V_sb write-before-read, and no
                    # psum_ot interleave with a live AvRetire accumulator)
                    urgent_q.extend([vproj_job(st, vT_sb, Vv) for st in range(8)])
                pending.append((pt, lc, *emit_pair(pt, lc)))
                if (pt, lc) == (1, 1):
                    # (3,0) was retired during this pair: half 0 is ready
                    spare_q.extend([out_lt_job(lt) for lt in range(4)])
            # tail: retire the second-to-last pair, flush spares (PE work
            # covering the last pair's exp drain), retire the last pair, then
            # half 1 (carried into the next batch's stream when one follows).
            push_retire(pending.pop(0))
            while urgent_q:
                urgent_q.pop(0)()
            while spare_q:
                spare_q.pop(0)()
            push_retire(pending.pop(0))
            while urgent_q:
                urgent_q.pop(0)()
            if b == BPC - 1:
                for lt in range(4, 8):
                    out_lt_job(lt)()
            else:
                spare_q.extend([out_lt_job(lt) for lt in range(4, 8)])

    nc.compile()
    _CACHE["nc"] = nc
    return nc


def _in_maps(inputs):
    import ml_dtypes

    e4 = ml_dtypes.float8_e4m3fn
    f = lambda a: np.ascontiguousarray(np.asarray(a, dtype=np.float32))
    queries, keys, values = f(inputs["queries"]), f(inputs["keys"]), f(inputs["values"])
    Wq, Wk, Wv, Wo = f(inputs["Wq"]), f(inputs["Wk"]), f(inputs["Wv"]), f(inputs["Wo"])
    bq, bk, bv, bo = f(inputs["bq"]), f(inputs["bk"]), f(inputs["bv"]), f(inputs["bo"])

    def w_dr(W):
        # [D, N] -> [g, p, k, N] fp8 with d = g*256 + k*128 + p, x16 prescale
        return np.ascontiguousarray(
            (W * WSCALE).reshape(2, 2, 128, -1).transpose(0, 2, 1, 3).astype(e4)
        )

    def x_dr(x):
        # [b, L, D] -> [b, g, p, k, L] fp8 with d = g*256 + k*128 + p
        xt = x.transpose(0, 2, 1)  # [b, D, L]
        bsz = xt.shape[0]
        return np.ascontiguousarray(
            xt.reshape(bsz, 2, 2, 128, -1).transpose(0, 1, 3, 2, 4).astype(e4)
        )

    b16 = ml_dtypes.bfloat16
    shared = {
        "Wq": w_dr(Wq), "Wk": w_dr(Wk), "Wv": np.ascontiguousarray(Wv.astype(b16)),
        "Wo": Wo,
        "bq_col": np.ascontiguousarray(bq.reshape(4, 128).T) * WSCALE,
        "bk_col": np.ascontiguousarray(bk.reshape(4, 128).T) * WSCALE,
        "bv_row": bv, "bo_row": bo,
    }
    maps = []
    for c in range(NCORES):
        sl = slice(BPC * c, BPC * (c + 1))
        maps.append({
            "qT": x_dr(queries[sl]),
            "kT": x_dr(keys[sl]),
            "vT": np.ascontiguousarray(values[sl].transpose(0, 2, 1).astype(b16)),
            **shared,
        })
    return maps


def kernel(**inputs) -> np.ndarray:
    global LAST_RESULTS
    from concourse import bass_utils

    nc = _build()
    maps = _in_maps(inputs)
    res = bass_utils.run_bass_kernel_spmd(nc, maps, core_ids=list(range(NCORES)))
    LAST_RESULTS = res
    return np.concatenate([res.results[c]["out"] for c in range(NCORES)], axis=0)


# revision 23
# speedup vs baseline: 1.1133x; 1.0944x over previous
"""Multi-head attention (B=16, L=S=1024, D=P=512, H=8) on 8 TRN2 NeuronCores.

Strategy: pure data parallelism over the batch — each core computes the full
attention block for 2 batch elements.  Activations are fed to the device
pre-transposed ([D, L] instead of [L, D]) so every GEMM contracts over the
partition dimension with no on-chip transposes:

  per batch element b (all on one core):
    QT[P,L] = Wq.T @ qT + bq   bf16 inputs, fp16 result
    KT[P,S] = Wk.T @ kT + bk   (fp16 so the K=64-contraction scores matmuls
                                run 1 cycle/col; f32r ran at 2 cycles/col)
    V [S,P] = vT.T @ Wv + bv   bf16 inputs, fp16 result in 128-wide head
                               blocks [1.0, 0*63, V_h] so the A@V matmul
                               emits softmax sums at psum partition 0 and the
                               head output at partitions 64..127 (both legal
                               PSUM offsets; the fast-recip custom-DVE op
                               reads sums straight from PSUM partition 0)
    per head h (E=64), software-pipelined two (h, L-chunk) chunks deep:
      expT[S,L] = exp(scale * K_h @ Q_h^T)
          S-chunks 0-5 on ACT (table exp, fp16 out); chunks 6-7 on the DVE
          as a Schraudolph bit-trick (int16(A*score+B) read back as fp16,
          ~3% sawtooth; softmax normalization cancels uniform scale error) —
          the ACT engine alone (1 col/cycle @1.2GHz) cannot keep up with the
          PE, and GPSIMD has no PSUM access so DVE is the only helper
      OT_h[E,L] = V128_h.T @ expT; OT rows normalized by the fast recip
          (GpSimd broadcasts the [1,512] recip row to 64 partitions)
    out[L,D] = OT.T-contraction with Wo + bo, emitted per L-half as soon as
          that half's last head is normalized (shrinks the tail)

Scheduling notes (measured on HW):
  - psum pools: scores+projections share one 3-buffer [128,1024] pool slot
    set; A@V + V-proj share a 2-buffer [128,512] pool (8 banks total).  The
    third scores buffer removes exp->matmul backpressure stalls.
  - The first two chunks' scores are issued before the V projection so the
    exp pipeline is full when the attention loop starts.
  - Weights/activations load as per-dt tiles: dependency tracking is
    tile-granular, so fused tiles made the first matmul wait on all 4 DMAs.
  - fp8 DoubleRow was measured a wash: it does halve A@V streaming (2
    moving cols/cycle), but the denominator matmuls it displaces (ones
    column no longer fits the M<=64 stationary) cost exactly the saving,
    and per-element fp8 error pushed rel-err to ~1.8e-2 vs the 2e-2 gate.

Roofline: the PE streams ~391k columns/core at 1 col/cycle @2.4GHz = 165us;
this kernel measures ~207.5us HW exec (~89% PE occupancy incl. ~11us fixed
startup and ~6us drain/epilogue), rel err ~3.9e-3 (gate 2e-2).  The TRN2
clock throttles ~17% after sustained back-to-back runs; timings above are
cold-chip numbers.
"""

import numpy as np

B, L, S, D, P, H, E = 16, 1024, 1024, 512, 512, 8, 64
NCORES = 8
BPC = B // NCORES  # batch elements per core
SCALE = 1.0 / float(np.sqrt(E))
WSCALE = 16.0  # host premultiplier on Wq/Wk so fp8e4 stays in normal range
QKSCALE = SCALE / (WSCALE * WSCALE)  # scores carry x256 from the two prescales

# Schraudolph exp, fp16 flavor: the DVE computes v = A*score + B in fp32 and
# converts to int16; the bit pattern read back as fp16 is ~exp(scale*score)
# with ~3% sawtooth error (softmax normalization cancels any uniform scale
# error, only the sawtooth shape survives).
SCHRAUD_A = float(2**10 / np.log(2)) * QKSCALE
SCHRAUD_B = float((15 - 0.043677448) * 2**10)  # 15 = fp16 exponent bias
DVE_STS = frozenset((6, 7))  # which of the 8 S-chunks per (h,lc) go to DVE


def _exp_plan(sp):
    """Per score-psum pair sp (sts 2sp, 2sp+1): list of (j0, j1, on_dve) ops."""
    a = (sp * 2) in DVE_STS
    b = (sp * 2 + 1) in DVE_STS
    if a == b:
        return [(0, 2, a)]
    return [(0, 1, a), (1, 2, b)]

_CACHE = {}
LAST_RESULTS = None  # stashed BassKernelResults for test harness introspection


def _build():
    """Build (once) the Bass program executed identically on all 8 cores."""
    if "nc" in _CACHE:
        return _CACHE["nc"]

    from contextlib import ExitStack

    import concourse.bass as bass
    import concourse.mybir as mybir
    import concourse.tile as tile
    from concourse import bacc

    f32 = mybir.dt.float32
    f32r = mybir.dt.float32r
    f16 = mybir.dt.float16
    i16 = mybir.dt.int16
    bf16 = mybir.dt.bfloat16
    fp8 = mybir.dt.float8e4
    AF = mybir.ActivationFunctionType
    ALU = mybir.AluOpType
    DR = mybir.MatmulPerfMode.DoubleRow

    nc = bacc.Bacc("TRN2", target_bir_lowering=False, debug=False)

    fp8 = mybir.dt.float8e4
    qT = nc.dram_tensor("qT", [BPC, 2, 128, 2, L], fp8, kind="ExternalInput").ap()
    kT = nc.dram_tensor("kT", [BPC, 2, 128, 2, S], fp8, kind="ExternalInput").ap()
    vT = nc.dram_tensor("vT", [BPC, D, S], bf16, kind="ExternalInput").ap()
    Wq = nc.dram_tensor("Wq", [2, 128, 2, P], fp8, kind="ExternalInput").ap()
    Wk = nc.dram_tensor("Wk", [2, 128, 2, P], fp8, kind="ExternalInput").ap()
    Wv = nc.dram_tensor("Wv", [D, P], bf16, kind="ExternalInput").ap()
    Wo = nc.dram_tensor("Wo", [P, D], f32, kind="ExternalInput").ap()
    bq_col = nc.dram_tensor("bq_col", [128, 4], f32, kind="ExternalInput").ap()
    bk_col = nc.dram_tensor("bk_col", [128, 4], f32, kind="ExternalInput").ap()
    bv_row = nc.dram_tensor("bv_row", [P], f32, kind="ExternalInput").ap()
    bo_row = nc.dram_tensor("bo_row", [D], f32, kind="ExternalInput").ap()
    ones_in = nc.dram_tensor("ones_in", [128, 128], f32, kind="ExternalInput").ap()
    out = nc.dram_tensor("out", [BPC, L, D], f32, kind="ExternalOutput").ap()

    def bcast_ap(src, n=128):
        # [N] DRAM vector (or [1, N] SBUF row) -> [n, N] partition-broadcast AP
        return bass.AP(tensor=src.tensor, offset=src.offset, ap=[[0, n]] + src.ap[-1:])

    with tile.TileContext(nc) as tc, ExitStack() as ctx:
        consts = ctx.enter_context(tc.tile_pool(name="consts", bufs=1))
        xT_pool = ctx.enter_context(tc.tile_pool(name="xT", bufs=2))
        acts = ctx.enter_context(tc.tile_pool(name="acts", bufs=1))
        exp_pool = ctx.enter_context(tc.tile_pool(name="exp", bufs=4))
        small = ctx.enter_context(tc.tile_pool(name="small", bufs=2))
        out_pool = ctx.enter_context(tc.tile_pool(name="outp", bufs=3))
        psum = ctx.enter_context(tc.tile_pool(name="psum", bufs=3, space="PSUM"))
        psum_ot = ctx.enter_context(tc.tile_pool(name="psum_ot", bufs=2, space="PSUM"))

        # ---- constants: weights [128, dtile, N] with contraction dim on partitions.
        # DMA issue order is interleaved with the first batch's activation loads
        # below so the first projection matmul isn't queued behind the weights.
        # per-dt tiles: tile-granular dependency tracking means a matmul on
        # dt=0 would otherwise wait for all four dt DMAs of a fused tile
        Wq_sb = [consts.tile([128, 2, P], fp8, tag=f"Wq{g}", name=f"Wq{g}") for g in range(2)]
        Wk_sb = [consts.tile([128, 2, P], fp8, tag=f"Wk{g}", name=f"Wk{g}") for g in range(2)]
        Wv_sb = [consts.tile([128, P], bf16, tag=f"Wv{dt}", name=f"Wv{dt}") for dt in range(4)]
        Wo_sb = [consts.tile([128, D], f32r, tag=f"Wo{dt}", name=f"Wo{dt}") for dt in range(4)]
        bq_sb = consts.tile([128, 4], f32, tag="bq")
        bk_sb = consts.tile([128, 4], f32, tag="bk")
        bv_sb = consts.tile([128, P], f32, tag="bv")
        bo_sb = consts.tile([128, D], f32, tag="bo")

        # V in 128-wide head blocks: col h*128 = 1.0, cols +1..63 = 0, cols
        # +64..127 = head h of V.  The OT matmul's [128,128] stationary then
        # emits the softmax denominator at psum PARTITION 0 (ones column) and
        # the head output at partitions 64..127 -- both PSUM-aligned offsets,
        # so the fast-recip custom-DVE op reads the sums straight from PSUM
        # (nonzero psum partition offsets trip a HW bug in custom-DVE ops).
        V_sb = consts.tile([128, 8, 8 * 128], f16, tag="V")  # [S-part, stile, 1024]
        Vv = V_sb.rearrange("p s (h e) -> p s h e", e=128)
        nc.vector.memset(Vv[:, :, :, 1:64], 0.0)
        nc.vector.memset(Vv[:, :, :, 0:1], 1.0)

        # PE warm-up: dummy matmuls over a small GpSimd-memset tile while the
        # first weight/activation DMAs are still in flight (GpSimd is idle at
        # ~5.7us; the DVE's big V-padding memset only lands at ~8-11us).  The
        # tensor engine needs ~3us of continuous execution to ramp from its
        # low power-state clock to 2.4GHz; without this the first ~30 real
        # matmuls run 1.3-2.7x slow (+3us measured).
        warm_sb = consts.tile([128, 512], f16, tag="warm")
        nc.gpsimd.memset(warm_sb, 1.0)
        warm_ps = psum.tile([128, 1024], f32, tag="scores", name="warm_ps")
        for _ in range(8):
            nc.tensor.matmul(
                warm_ps[0:64, 0:512],
                warm_sb[:, 0:64],
                warm_sb,
                start=True,
                stop=True,
            )
        # engine warm-ups in the DMA dead zone: the first Identity/Exp on the
        # ACT triggers an activation-table load and the first
        # PartitionBroadcast on GPSIMD pays a ~6us IRAM library load.
        wf32 = consts.tile([1, 16], f32, tag="wf32")
        wfid = consts.tile([1, 16], f16, tag="wfid")
        wfex = consts.tile([1, 16], f16, tag="wfex")
        wfsc = consts.tile([1, 16], f16, tag="wfsc")
        wfrc = consts.tile([1, 16], f32, tag="wfrc")
        wfbc = consts.tile([16, 16], f32, tag="wfbc")
        nc.vector.memset(wf32, 1.0)
        nc.scalar.activation(out=wfid, in_=wf32, func=AF.Identity,
                             bias=wf32[:, 0:1])
        nc.scalar.activation(out=wfex, in_=wf32, func=AF.Exp, scale=QKSCALE)
        nc.vector.tensor_scalar(out=wfsc.bitcast(i16), in0=wf32,
                                scalar1=SCHRAUD_A, scalar2=SCHRAUD_B,
                                op0=ALU.mult, op1=ALU.add)
        nc.vector.reciprocal_approx_fast(out=wfrc, in_=wf32)
        nc.gpsimd.partition_broadcast(wfbc, wfrc, channels=16)

        def load_xT8(src, b, name):
            ts = [xT_pool.tile([128, 2, L], fp8, tag=f"{name}{g}", name=f"{name}{g}")
                  for g in range(2)]
            for g in range(2):
                nc.sync.dma_start(out=ts[g], in_=src[b, g])
            return ts

        def load_xT(src, b, name, dtype):
            # per-dt tiles + DMAs so each projection matmul waits only its dt
            ts = [xT_pool.tile([128, L], dtype, tag=f"{name}{dt}", name=f"{name}{dt}")
                  for dt in range(4)]
            view = src[b].rearrange("(t p) l -> p t l", p=128)
            for dt in range(4):
                nc.sync.dma_start(out=ts[dt], in_=view[:, dt, :])
            return ts

        def load_w(W_sb, Wsrc, dtype):
            view = Wsrc.rearrange("(t p) n -> p t n", p=128)
            if dtype == f32r:
                view = view.bitcast(f32r)
            for dt in range(4):
                nc.sync.dma_start(out=W_sb[dt], in_=view[:, dt, :])

        # The first psum group consumes (Wq[dt], qT[dt]) in dt order: issue the
        # DMAs in exactly that order, alternating across the sync and gpsimd
        # queues so transfers overlap.
        qT0_sb = [xT_pool.tile([128, 2, L], fp8, tag=f"qT_sb{g}", name=f"qT0_{g}")
                  for g in range(2)]
        nc.scalar.dma_start(out=Wq_sb[0], in_=Wq[0])
        nc.gpsimd.dma_start(out=qT0_sb[0], in_=qT[0, 0])
        nc.sync.dma_start(out=Wq_sb[1], in_=Wq[1])
        nc.gpsimd.dma_start(out=qT0_sb[1], in_=qT[0, 1])
        first = {"qT_sb": qT0_sb}
        nc.sync.dma_start(out=bq_sb, in_=bq_col)
        for g in range(2):
            nc.sync.dma_start(out=Wk_sb[g], in_=Wk[g])
        nc.sync.dma_start(out=bk_sb, in_=bk_col)
        first["kT_sb"] = load_xT8(kT, 0, "kT_sb")
        load_w(Wv_sb, Wv, bf16)
        nc.gpsimd.dma_start(out=bv_sb, in_=bcast_ap(bv_row))
        first["vT_sb"] = load_xT(vT, 0, "vT_sb", bf16)
        load_w(Wo_sb, Wo, f32r)
        nc.gpsimd.dma_start(out=bo_sb, in_=bcast_ap(bo_row))

        for b in range(BPC):
            if b == 0:
                qT_sb, kT_sb, vT_sb = first["qT_sb"], first["kT_sb"], first["vT_sb"]
            else:
                qT_sb = load_xT8(qT, b, "qT_sb")
                kT_sb = load_xT8(kT, b, "kT_sb")
                vT_sb = load_xT(vT, b, "vT_sb", bf16)

            QT_sb = acts.tile([128, 4, L], f16, tag="QT")  # [P-part, ptile, L]
            KT_sb = acts.tile([128, 4, S], f16, tag="KT")
            OT_sb = acts.tile([128, 4, L], f32r, tag="OT")  # [P-part, ptile, L]

            # ---- QT / KT projections, fp8 DoubleRow:
            # psum[p, l] = sum_d W[d, p] * xT[d, l]; W carries a x16 host
            # prescale (fp8e4 range), undone in the bias tensor_scalar.
            for W_sb, b_sb, X_sb, Y_sb in (
                (Wq_sb, bq_sb, qT_sb, QT_sb),
                (Wk_sb, bk_sb, kT_sb, KT_sb),
            ):
                for pt in range(4):
                    for lc in range(2):
                        ps = psum.tile([128, 1024], f32, tag="scores", name="ps")[:, 0:512]
                        for g in range(2):
                            nc.tensor.matmul(
                                ps,
                                W_sb[g][:, :, pt * 128:(pt + 1) * 128],
                                X_sb[g][:, :, lc * 512:(lc + 1) * 512],
                                start=(g == 0),
                                stop=(g == 1),
                                perf_mode=mybir.MatmulPerfMode.DoubleRow,
                            )
                        nc.scalar.activation(
                            out=Y_sb[:, pt, lc * 512:(lc + 1) * 512],
                            in_=ps,
                            func=AF.Identity,
                            bias=b_sb[:, pt:pt + 1],
                        )

            # ---- attention, software-pipelined one (head, L-chunk) deep so the
            # PE runs scores(c) while ACT/GpSimd still exponentiate chunk c-1.
            def emit_scores_half(h, lc, expT_c, sps):
                pt_h, po_h = h // 2, (h % 2) * 64
                lsl = slice(lc * 512, (lc + 1) * 512)
                for sp in sps:
                    ps_s = psum.tile([128, 1024], f32, tag="scores", name="ps_s")
                    for j in range(2):
                        st = sp * 2 + j
                        nc.tensor.matmul(
                            ps_s[:, j * 512:(j + 1) * 512],
                            KT_sb[po_h:po_h + 64, pt_h, st * 128:(st + 1) * 128],
                            QT_sb[po_h:po_h + 64, pt_h, lsl],
                            start=True,
                            stop=True,
                        )
                    # exp split: ACT does sts 0-4 (table exp, consumed first
                    # by the OT matmuls), DVE does sts 5-7 (Schraudolph: int16
                    # bits <- scores*(A*scale)+B, read back as fp16 exp; DVE
                    # not GpSimd because GPSIMD has no PSUM access on HW).
                    psv = ps_s.rearrange("p (a b) -> p a b", b=512)
                    for j0, j1, on_dve in _exp_plan(sp):
                        if on_dve:
                            nc.vector.tensor_scalar(
                                out=expT_c[:, sp * 2 + j0:sp * 2 + j1, :].bitcast(i16),
                                in0=psv[:, j0:j1, :],
                                scalar1=SCHRAUD_A,
                                scalar2=SCHRAUD_B,
                                op0=ALU.mult,
                                op1=ALU.add,
                            )
                        else:
                            nc.scalar.activation(
                                out=expT_c[:, sp * 2 + j0:sp * 2 + j1, :],
                                in_=psv[:, j0:j1, :],
                                func=AF.Exp,
                                scale=QKSCALE,
                            )

            def emit_ot(h, lc, expT_c, ps_o):
                for st in range(8):
                    nc.tensor.matmul(
                        ps_o,
                        V_sb[:, st, h * 128:(h + 1) * 128],
                        expT_c[:, st, :],
                        start=(st == 0),
                        stop=(st == 7),
                    )

            def emit_norm(h, lc, ps_o):
                pt_h, po_h = h // 2, (h % 2) * 64
                lsl = slice(lc * 512, (lc + 1) * 512)
                recip_sb = small.tile([1, 512], f32, tag="recip", name="recip_sb")
                nc.vector.reciprocal_approx_fast(out=recip_sb, in_=ps_o[0:1, :])
                rep_sb = small.tile([64, 512], f32, tag="rep", name="rep_sb")
                nc.gpsimd.partition_broadcast(rep_sb, recip_sb, channels=64)
                nc.vector.tensor_mul(
                    OT_sb[po_h:po_h + 64, pt_h, lsl], ps_o[64:128, :], rep_sb
                )


            def emit_chunk(h, lc):
                expT_c = exp_pool.tile([128, 8, 512], f16, tag="expT", name="expT_c")
                emit_scores_half(h, lc, expT_c, (0, 1))
                emit_scores_half(h, lc, expT_c, (2, 3))
                return expT_c

            # prime the exp pipeline: the first two chunks' scores issue
            # before the V projection (whose psum comes from the ot pool, so
            # it does not couple to these tiles' exp completions); their
            # exponentials finish while the PE runs the V projection.
            primed = [(h, 0, emit_chunk(h, 0)) for h in range(2)]

            # ---- V projection (bf16): psum[s, p] = sum_d vT[d, s] * Wv[d, p]
            for st in range(8):
                ps = psum_ot.tile([128, 512], f32, tag="ot", name="ps")
                for dt in range(4):
                    nc.tensor.matmul(
                        ps,
                        vT_sb[dt][:, st * 128:(st + 1) * 128],
                        Wv_sb[dt],
                        start=(dt == 0),
                        stop=(dt == 3),
                    )
                nc.vector.tensor_add(
                    Vv[:, st, :, 64:128],
                    ps.rearrange("p (h e) -> p h e", e=64),
                    bv_sb.rearrange("p (h e) -> p h e", e=64),
                )

            def emit_out_proj_half(lc):
                # out projection for l rows lc*512..lc*512+511 (needs all heads
                # of that L-half in OT_sb): psum[l, d] = sum_p OT[p,l]*Wo[p,d]
                for lt in range(lc * 4, lc * 4 + 4):
                    ps = psum.tile([128, 1024], f32, tag="scores", name="ps")[:, 0:512]
                    for pt in range(4):
                        nc.tensor.matmul(
                            ps,
                            OT_sb[:, pt, lt * 128:(lt + 1) * 128],
                            Wo_sb[pt],
                            start=(pt == 0),
                            stop=(pt == 3),
                        )
                    o_sb = out_pool.tile([128, 512], f32, tag="osb")
                    nc.vector.tensor_add(o_sb, ps, bo_sb)
                    nc.sync.dma_start(out=out[b, lt * 128:(lt + 1) * 128, :], in_=o_sb)

            def pop_pending():
                ph, plc, pexp = pending.pop(0)
                ps_o = psum_ot.tile([128, 512], f32, tag="ot", name="ps_o")
                emit_ot(ph, plc, pexp, ps_o)
                emit_norm(ph, plc, ps_o)
                if (ph, plc) == (H - 1, 0):
                    emit_out_proj_half(0)  # all lc=0 heads normalized
                return plc

            pending = list(primed)
            for h in range(H):
                for lc in range(2):
                    if lc == 0 and h < len(primed):
                        continue  # scores already issued before the V proj
                    pending.append((h, lc, emit_chunk(h, lc)))
                    if len(pending) > 2:  # 2-deep stagger: OT runs two chunks behind
                        pop_pending()
            while pending:
                pop_pending()
            emit_out_proj_half(1)

    nc.compile()
    _CACHE["nc"] = nc
    return nc


def _in_maps(inputs):
    import ml_dtypes

    b16 = ml_dtypes.bfloat16
    e4 = ml_dtypes.float8_e4m3fn
    f = lambda a: np.ascontiguousarray(np.asarray(a, dtype=np.float32))
    queries, keys, values = f(inputs["queries"]), f(inputs["keys"]), f(inputs["values"])
    Wq, Wk, Wv, Wo = f(inputs["Wq"]), f(inputs["Wk"]), f(inputs["Wv"]), f(inputs["Wo"])
    bq, bk, bv, bo = f(inputs["bq"]), f(inputs["bk"]), f(inputs["bv"]), f(inputs["bo"])

    def w_dr(W):
        return np.ascontiguousarray(
            (W * WSCALE).reshape(2, 2, 128, -1).transpose(0, 2, 1, 3).astype(e4)
        )

    def x_dr(x):
        xt = x.transpose(0, 2, 1)
        bsz = xt.shape[0]
        return np.ascontiguousarray(
            xt.reshape(bsz, 2, 2, 128, -1).transpose(0, 1, 3, 2, 4).astype(e4)
        )

    shared = {
        "Wq": w_dr(Wq), "Wk": w_dr(Wk),
        "Wv": np.ascontiguousarray(Wv.astype(b16)), "Wo": Wo,
        "bq_col": np.ascontiguousarray(bq.reshape(4, 128).T) * WSCALE,
        "bk_col": np.ascontiguousarray(bk.reshape(4, 128).T) * WSCALE,
        "bv_row": bv, "bo_row": bo,
        "ones_in": np.ones((128, 128), np.float32),
    }
    maps = []
    for c in range(NCORES):
        sl = slice(BPC * c, BPC * (c + 1))
        maps.append({
            "qT": x_dr(queries[sl]),
            "kT": x_dr(keys[sl]),
            "vT": np.ascontiguousarray(values[sl].transpose(0, 2, 1).astype(b16)),
            **shared,
        })
    return maps


def kernel(**inputs) -> np.ndarray:
    global LAST_RESULTS
    from concourse import bass_utils

    nc = _build()
    maps = _in_maps(inputs)
    res = bass_utils.run_bass_kernel_spmd(nc, maps, core_ids=list(range(NCORES)))
    LAST_RESULTS = res
    return np.concatenate([res.results[c]["out"] for c in range(NCORES)], axis=0)



# revision 26
# speedup vs baseline: 1.1317x; 1.0165x over previous
"""Multi-head attention (B=16, L=S=1024, D=P=512, H=8) on 8 TRN2 NeuronCores.

Strategy: pure data parallelism over the batch — each core computes the full
attention block for 2 batch elements.  Activations are fed to the device
pre-transposed ([D, L] instead of [L, D]) so every GEMM contracts over the
partition dimension with no on-chip transposes:

  per batch element b (all on one core):
    QT[P,L] = Wq.T @ qT + 16bq fp8e4 DoubleRow: host interleaves (d -> g,p,k
    KT[P,S] = Wk.T @ kT + 16bk  = g*256+k*128+p) so one DR matmul contracts
                                256 rows -- 2 matmuls instead of 4 per psum
                                (HW-measured ~234ns vs 2x216ns).  Weights
                                carry a x16 host prescale to keep fp8e4 in
                                normal range; the resulting x256 on scores
                                folds into the exp scale.  fp8 is safe HERE
                                because softmax normalization absorbs the
                                ~1% score error (fp8 on the V path measured
                                1.3e-2 rel err and was reverted).  The
                                bias+copy drains alternate ACT (Identity
                                with per-partition bias AP) / DVE so the
                                3-buf psum pool keeps up with DR production.
    V [S,P] = vT.T @ Wv + bv   bf16 inputs, fp16 result in 128-wide head
                               blocks [1.0, 0*63, V_h] so the A@V matmul
                               emits softmax sums at psum partition 0 and the
                               head output at partitions 64..127 (both legal
                               PSUM offsets; the fast-recip custom-DVE op
                               reads sums straight from PSUM partition 0)
    per head h (E=64), software-pipelined two (h, L-chunk) chunks deep:
      expT[S,L] = exp(scale * K_h @ Q_h^T)
          S-chunks 0-5 on ACT (table exp, fp16 out); chunks 6-7 on the DVE
          as a Schraudolph bit-trick (int16(A*score+B) read back as fp16,
          ~3% sawtooth; softmax normalization cancels uniform scale error) —
          the ACT engine alone (1 col/cycle @1.2GHz) cannot keep up with the
          PE, and GPSIMD has no PSUM access so DVE is the only helper
      OT_h[E,L] = V128_h.T @ expT; OT rows normalized by the fast recip
          (GpSimd broadcasts the [1,512] recip row to 64 partitions)
    out[L,D] = OT.T-contraction with Wo + bo, emitted per L-half as soon as
          that half's last head is normalized (shrinks the tail)

Scheduling notes (measured on HW):
  - psum pools: scores+projections share one 3-buffer [128,1024] pool slot
    set; A@V + V-proj share a 2-buffer [128,512] pool (8 banks total).  The
    third scores buffer removes exp->matmul backpressure stalls.
  - The first two chunks' scores are issued before the V projection so the
    exp pipeline is full when the attention loop starts.
  - Weights/activations load as per-dt tiles: dependency tracking is
    tile-granular, so fused tiles made the first matmul wait on all 4 DMAs.
  - Engine warm-ups in the DMA dead zone: the first ACT Identity/Exp pays an
    activation-table load and the first GpSimd PartitionBroadcast a ~6us
    IRAM library load; tiny dummies at t~0 hide both.
  - fp8 DoubleRow on A@V was measured a wash (the displaced ones-column
    denominator matmuls cost the saving back) and fp8 expT/V pushed rel-err
    near the gate; only the Q/K projections keep fp8.
  - Row-tile pairing of the K=64 score matmuls (even head on T0, odd on T8)
    reaches 121ns/MM in a clean microbench, but in-kernel the scores psum
    pool (3x[128,1024] of 8 banks) WARs against the ACT/DVE exp queue
    latency and the pairing never engaged at scale -- four restructures
    (per-sp weave, mid-pair A@V blocks, 2-deep retirement, filler-job
    queues) all measured 220-268us vs this layout's 200us.

Roofline: the PE streams ~344k columns/core (scores+A@V 262k at 1 col/cycle,
QKV fp8-DR projections 2 cols/cycle-equivalent) = ~149us @2.4GHz; this
kernel measures ~201us HW exec (PE busy ~177us incl LDW/mode overheads,
~7.7us gaps, 7.8us DMA head, 13us tail), rel err ~5.9e-3 (gate 2e-2).  The
TRN2 clock throttles ~17% after sustained back-to-back runs; timings above
are cold-chip numbers.
"""

import numpy as np

B, L, S, D, P, H, E = 16, 1024, 1024, 512, 512, 8, 64
NCORES = 8
BPC = B // NCORES  # batch elements per core
SCALE = 1.0 / float(np.sqrt(E))
WSCALE = 16.0  # host premultiplier on Wq/Wk so fp8e4 stays in normal range
QKSCALE = SCALE / (WSCALE * WSCALE)  # scores carry x256 from the two prescales

# Schraudolph exp, fp16 flavor: the DVE computes v = A*score + B in fp32 and
# converts to int16; the bit pattern read back as fp16 is ~exp(scale*score)
# with ~3% sawtooth error (softmax normalization cancels any uniform scale
# error, only the sawtooth shape survives).
SCHRAUD_A = float(2**10 / np.log(2)) * QKSCALE
SCHRAUD_B = float((15 - 0.043677448) * 2**10)  # 15 = fp16 exponent bias
DVE_STS = frozenset((6, 7))  # which of the 8 S-chunks per (h,lc) go to DVE


def _exp_plan(sp):
    """Per score-psum pair sp (sts 2sp, 2sp+1): list of (j0, j1, on_dve) ops."""
    a = (sp * 2) in DVE_STS
    b = (sp * 2 + 1) in DVE_STS
    if a == b:
        return [(0, 2, a)]
    return [(0, 1, a), (1, 2, b)]

_CACHE = {}
LAST_RESULTS = None  # stashed BassKernelResults for test harness introspection


def _build():
    """Build (once) the Bass program executed identically on all 8 cores."""
    if "nc" in _CACHE:
        return _CACHE["nc"]

    from contextlib import ExitStack

    import concourse.bass as bass
    import concourse.mybir as mybir
    import concourse.tile as tile
    from concourse import bacc

    f32 = mybir.dt.float32
    f32r = mybir.dt.float32r
    f16 = mybir.dt.float16
    i16 = mybir.dt.int16
    bf16 = mybir.dt.bfloat16
    fp8 = mybir.dt.float8e4
    AF = mybir.ActivationFunctionType
    ALU = mybir.AluOpType
    DR = mybir.MatmulPerfMode.DoubleRow

    nc = bacc.Bacc("TRN2", target_bir_lowering=False, debug=False)

    fp8 = mybir.dt.float8e4
    qT = nc.dram_tensor("qT", [BPC, 2, 128, 2, L], fp8, kind="ExternalInput").ap()
    kT = nc.dram_tensor("kT", [BPC, 2, 128, 2, S], fp8, kind="ExternalInput").ap()
    vT = nc.dram_tensor("vT", [BPC, D, S], bf16, kind="ExternalInput").ap()
    Wq = nc.dram_tensor("Wq", [2, 128, 2, P], fp8, kind="ExternalInput").ap()
    Wk = nc.dram_tensor("Wk", [2, 128, 2, P], fp8, kind="ExternalInput").ap()
    Wv = nc.dram_tensor("Wv", [D, P], bf16, kind="ExternalInput").ap()
    Wo = nc.dram_tensor("Wo", [P, D], f32, kind="ExternalInput").ap()
    bq_col = nc.dram_tensor("bq_col", [128, 4], f32, kind="ExternalInput").ap()
    bk_col = nc.dram_tensor("bk_col", [128, 4], f32, kind="ExternalInput").ap()
    bv_row = nc.dram_tensor("bv_row", [P], f32, kind="ExternalInput").ap()
    bo_row = nc.dram_tensor("bo_row", [D], f32, kind="ExternalInput").ap()
    ones_in = nc.dram_tensor("ones_in", [128, 128], f32, kind="ExternalInput").ap()
    out = nc.dram_tensor("out", [BPC, L, D], f32, kind="ExternalOutput").ap()

    def bcast_ap(src, n=128):
        # [N] DRAM vector (or [1, N] SBUF row) -> [n, N] partition-broadcast AP
        return bass.AP(tensor=src.tensor, offset=src.offset, ap=[[0, n]] + src.ap[-1:])

    with tile.TileContext(nc) as tc, ExitStack() as ctx:
        consts = ctx.enter_context(tc.tile_pool(name="consts", bufs=1))
        xT_pool = ctx.enter_context(tc.tile_pool(name="xT", bufs=2))
        acts = ctx.enter_context(tc.tile_pool(name="acts", bufs=1))
        exp_pool = ctx.enter_context(tc.tile_pool(name="exp", bufs=4))
        small = ctx.enter_context(tc.tile_pool(name="small", bufs=2))
        out_pool = ctx.enter_context(tc.tile_pool(name="outp", bufs=3))
        psum = ctx.enter_context(tc.tile_pool(name="psum", bufs=3, space="PSUM"))
        psum_ot = ctx.enter_context(tc.tile_pool(name="psum_ot", bufs=2, space="PSUM"))

        # ---- constants: weights [128, dtile, N] with contraction dim on partitions.
        # DMA issue order is interleaved with the first batch's activation loads
        # below so the first projection matmul isn't queued behind the weights.
        # per-dt tiles: tile-granular dependency tracking means a matmul on
        # dt=0 would otherwise wait for all four dt DMAs of a fused tile
        Wq_sb = [consts.tile([128, 2, P], fp8, tag=f"Wq{g}", name=f"Wq{g}") for g in range(2)]
        Wk_sb = [consts.tile([128, 2, P], fp8, tag=f"Wk{g}", name=f"Wk{g}") for g in range(2)]
        Wv_sb = [consts.tile([128, P], bf16, tag=f"Wv{dt}", name=f"Wv{dt}") for dt in range(4)]
        Wo_sb = [consts.tile([128, D], f32r, tag=f"Wo{dt}", name=f"Wo{dt}") for dt in range(4)]
        bq_sb = consts.tile([128, 4], f32, tag="bq")
        bk_sb = consts.tile([128, 4], f32, tag="bk")
        bv_sb = consts.tile([128, P], f32, tag="bv")
        bo_sb = consts.tile([128, D], f32, tag="bo")

        # V in 128-wide head blocks: col h*128 = 1.0, cols +1..63 = 0, cols
        # +64..127 = head h of V.  The OT matmul's [128,128] stationary then
        # emits the softmax denominator at psum PARTITION 0 (ones column) and
        # the head output at partitions 64..127 -- both PSUM-aligned offsets,
        # so the fast-recip custom-DVE op reads the sums straight from PSUM
        # (nonzero psum partition offsets trip a HW bug in custom-DVE ops).
        V_sb = consts.tile([128, 8, 8 * 128], f16, tag="V")  # [S-part, stile, 1024]
        Vv = V_sb.rearrange("p s (h e) -> p s h e", e=128)
        nc.vector.memset(Vv[:, :, :, 1:64], 0.0)
        nc.vector.memset(Vv[:, :, :, 0:1], 1.0)

        # PE warm-up: dummy matmuls over a small GpSimd-memset tile while the
        # first weight/activation DMAs are still in flight (GpSimd is idle at
        # ~5.7us; the DVE's big V-padding memset only lands at ~8-11us).  The
        # tensor engine needs ~3us of continuous execution to ramp from its
        # low power-state clock to 2.4GHz; without this the first ~30 real
        # matmuls run 1.3-2.7x slow (+3us measured).
        warm_sb = consts.tile([128, 512], f16, tag="warm")
        nc.gpsimd.memset(warm_sb, 1.0)
        warm_ps = psum.tile([128, 1024], f32, tag="scores", name="warm_ps")
        for _ in range(8):
            nc.tensor.matmul(
                warm_ps[0:64, 0:512],
                warm_sb[:, 0:64],
                warm_sb,
                start=True,
                stop=True,
            )
        # engine warm-ups in the DMA dead zone: the first Identity/Exp on the
        # ACT triggers an activation-table load and the first
        # PartitionBroadcast on GPSIMD pays a ~6us IRAM library load.
        wf32 = consts.tile([1, 16], f32, tag="wf32")
        wfid = consts.tile([1, 16], f16, tag="wfid")
        wfex = consts.tile([1, 16], f16, tag="wfex")
        wfsc = consts.tile([1, 16], f16, tag="wfsc")
        wfrc = consts.tile([1, 16], f32, tag="wfrc")
        wfbc = consts.tile([16, 16], f32, tag="wfbc")
        nc.vector.memset(wf32, 1.0)
        nc.scalar.activation(out=wfid, in_=wf32, func=AF.Identity,
                             bias=wf32[:, 0:1])
        nc.scalar.activation(out=wfex, in_=wf32, func=AF.Exp, scale=QKSCALE)
        nc.vector.tensor_scalar(out=wfsc.bitcast(i16), in0=wf32,
                                scalar1=SCHRAUD_A, scalar2=SCHRAUD_B,
                                op0=ALU.mult, op1=ALU.add)
        nc.vector.reciprocal_approx_fast(out=wfrc, in_=wf32)
        nc.gpsimd.partition_broadcast(wfbc, wfrc, channels=16)

        def load_xT8(src, b, name):
            ts = [xT_pool.tile([128, 2, L], fp8, tag=f"{name}{g}", name=f"{name}{g}")
                  for g in range(2)]
            for g in range(2):
                nc.sync.dma_start(out=ts[g], in_=src[b, g])
            return ts

        def load_xT(src, b, name, dtype):
            # per-dt tiles + DMAs so each projection matmul waits only its dt
            ts = [xT_pool.tile([128, L], dtype, tag=f"{name}{dt}", name=f"{name}{dt}")
                  for dt in range(4)]
            view = src[b].rearrange("(t p) l -> p t l", p=128)
            for dt in range(4):
                nc.sync.dma_start(out=ts[dt], in_=view[:, dt, :])
            return ts

        def load_w(W_sb, Wsrc, dtype):
            view = Wsrc.rearrange("(t p) n -> p t n", p=128)
            if dtype == f32r:
                view = view.bitcast(f32r)
            for dt in range(4):
                nc.sync.dma_start(out=W_sb[dt], in_=view[:, dt, :])

        # The first psum group consumes (Wq[dt], qT[dt]) in dt order: issue the
        # DMAs in exactly that order, alternating across the sync and gpsimd
        # queues so transfers overlap.
        qT0_sb = [xT_pool.tile([128, 2, L], fp8, tag=f"qT_sb{g}", name=f"qT0_{g}")
                  for g in range(2)]
        nc.scalar.dma_start(out=Wq_sb[0], in_=Wq[0])
        nc.gpsimd.dma_start(out=qT0_sb[0], in_=qT[0, 0])
        nc.sync.dma_start(out=Wq_sb[1], in_=Wq[1])
        nc.gpsimd.dma_start(out=qT0_sb[1], in_=qT[0, 1])
        first = {"qT_sb": qT0_sb}
        nc.sync.dma_start(out=bq_sb, in_=bq_col)
        for g in range(2):
            nc.sync.dma_start(out=Wk_sb[g], in_=Wk[g])
        nc.sync.dma_start(out=bk_sb, in_=bk_col)
        first["kT_sb"] = load_xT8(kT, 0, "kT_sb")
        load_w(Wv_sb, Wv, bf16)
        nc.gpsimd.dma_start(out=bv_sb, in_=bcast_ap(bv_row))
        first["vT_sb"] = load_xT(vT, 0, "vT_sb", bf16)
        load_w(Wo_sb, Wo, f32r)
        nc.gpsimd.dma_start(out=bo_sb, in_=bcast_ap(bo_row))

        for b in range(BPC):
            if b == 0:
                qT_sb, kT_sb, vT_sb = first["qT_sb"], first["kT_sb"], first["vT_sb"]
            else:
                qT_sb = load_xT8(qT, b, "qT_sb")
                kT_sb = load_xT8(kT, b, "kT_sb")
                vT_sb = load_xT(vT, b, "vT_sb", bf16)

            QT_sb = acts.tile([128, 4, L], f16, tag="QT")  # [P-part, ptile, L]
            KT_sb = acts.tile([128, 4, S], f16, tag="KT")
            OT_sb = acts.tile([128, 4, L], f32r, tag="OT")  # [P-part, ptile, L]

            # ---- QT / KT projections, fp8 DoubleRow:
            # psum[p, l] = sum_d W[d, p] * xT[d, l]; W carries a x16 host
            # prescale (fp8e4 range), undone in the bias tensor_scalar.
            for W_sb, b_sb, X_sb, Y_sb in (
                (Wq_sb, bq_sb, qT_sb, QT_sb),
                (Wk_sb, bk_sb, kT_sb, KT_sb),
            ):
                for pt in range(4):
                    for lc in range(2):
                        ps = psum.tile([128, 1024], f32, tag="scores", name="ps")[:, 0:512]
                        for g in range(2):
                            nc.tensor.matmul(
                                ps,
                                W_sb[g][:, :, pt * 128:(pt + 1) * 128],
                                X_sb[g][:, :, lc * 512:(lc + 1) * 512],
                                start=(g == 0),
                                stop=(g == 1),
                                perf_mode=mybir.MatmulPerfMode.DoubleRow,
                            )
                        # alternate the bias+copy between ACT and DVE so the
                        # psum pool drains on two engines in parallel (DR
                        # matmuls produce faster than one engine evacuates)
                        if (pt + lc) % 2 == 0:
                            nc.scalar.activation(
                                out=Y_sb[:, pt, lc * 512:(lc + 1) * 512],
                                in_=ps,
                                func=AF.Identity,
                                bias=b_sb[:, pt:pt + 1],
                            )
                        else:
                            nc.vector.tensor_scalar_add(
                                Y_sb[:, pt, lc * 512:(lc + 1) * 512],
                                ps,
                                b_sb[:, pt:pt + 1],
                            )

            # ---- attention, software-pipelined one (head, L-chunk) deep so the
            # PE runs scores(c) while ACT/GpSimd still exponentiate chunk c-1.
            def emit_scores_half(h, lc, expT_c, sps):
                pt_h, po_h = h // 2, (h % 2) * 64
                lsl = slice(lc * 512, (lc + 1) * 512)
                for sp in sps:
                    ps_s = psum.tile([128, 1024], f32, tag="scores", name="ps_s")
                    for j in range(2):
                        st = sp * 2 + j
                        nc.tensor.matmul(
                            ps_s[:, j * 512:(j + 1) * 512],
                            KT_sb[po_h:po_h + 64, pt_h, st * 128:(st + 1) * 128],
                            QT_sb[po_h:po_h + 64, pt_h, lsl],
                            start=True,
                            stop=True,
                        )
                    # exp split: ACT does sts 0-4 (table exp, consumed first
                    # by the OT matmuls), DVE does sts 5-7 (Schraudolph: int16
                    # bits <- scores*(A*scale)+B, read back as fp16 exp; DVE
                    # not GpSimd because GPSIMD has no PSUM access on HW).
                    psv = ps_s.rearrange("p (a b) -> p a b", b=512)
                    for j0, j1, on_dve in _exp_plan(sp):
                        if on_dve:
                            nc.vector.tensor_scalar(
                                out=expT_c[:, sp * 2 + j0:sp * 2 + j1, :].bitcast(i16),
                                in0=psv[:, j0:j1, :],
                                scalar1=SCHRAUD_A,
                                scalar2=SCHRAUD_B,
                                op0=ALU.mult,
                                op1=ALU.add,
                            )
                        else:
                            nc.scalar.activation(
                                out=expT_c[:, sp * 2 + j0:sp * 2 + j1, :],
                                in_=psv[:, j0:j1, :],
                                func=AF.Exp,
                                scale=QKSCALE,
                            )

            def emit_ot(h, lc, expT_c, ps_o):
                for st in range(8):
                    nc.tensor.matmul(
                        ps_o,
                        V_sb[:, st, h * 128:(h + 1) * 128],
                        expT_c[:, st, :],
                        start=(st == 0),
                        stop=(st == 7),
                    )

            def emit_norm(h, lc, ps_o):
                pt_h, po_h = h // 2, (h % 2) * 64
                lsl = slice(lc * 512, (lc + 1) * 512)
                recip_sb = small.tile([1, 512], f32, tag="recip", name="recip_sb")
                nc.vector.reciprocal_approx_fast(out=recip_sb, in_=ps_o[0:1, :])
                rep_sb = small.tile([64, 512], f32, tag="rep", name="rep_sb")
                nc.gpsimd.partition_broadcast(rep_sb, recip_sb, channels=64)
                nc.vector.tensor_mul(
                    OT_sb[po_h:po_h + 64, pt_h, lsl], ps_o[64:128, :], rep_sb
                )


            def emit_chunk(h, lc):
                expT_c = exp_pool.tile([128, 8, 512], f16, tag="expT", name="expT_c")
                emit_scores_half(h, lc, expT_c, (0, 1))
                emit_scores_half(h, lc, expT_c, (2, 3))
                return expT_c

            # prime the exp pipeline: the first two chunks' scores issue
            # before the V projection (whose psum comes from the ot pool, so
            # it does not couple to these tiles' exp completions); their
            # exponentials finish while the PE runs the V projection.
            primed = [(h, 0, emit_chunk(h, 0)) for h in range(2)]

            # ---- V projection (bf16): psum[s, p] = sum_d vT[d, s] * Wv[d, p]
            for st in range(8):
                ps = psum_ot.tile([128, 512], f32, tag="ot", name="ps")
                for dt in range(4):
                    nc.tensor.matmul(
                        ps,
                        vT_sb[dt][:, st * 128:(st + 1) * 128],
                        Wv_sb[dt],
                        start=(dt == 0),
                        stop=(dt == 3),
                    )
                nc.vector.tensor_add(
                    Vv[:, st, :, 64:128],
                    ps.rearrange("p (h e) -> p h e", e=64),
                    bv_sb.rearrange("p (h e) -> p h e", e=64),
                )

            def emit_out_proj_half(lc):
                # out projection for l rows lc*512..lc*512+511 (needs all heads
                # of that L-half in OT_sb): psum[l, d] = sum_p OT[p,l]*Wo[p,d]
                for lt in range(lc * 4, lc * 4 + 4):
                    ps = psum.tile([128, 1024], f32, tag="scores", name="ps")[:, 0:512]
                    for pt in range(4):
                        nc.tensor.matmul(
                            ps,
                            OT_sb[:, pt, lt * 128:(lt + 1) * 128],
                            Wo_sb[pt],
                            start=(pt == 0),
                            stop=(pt == 3),
                        )
                    o_sb = out_pool.tile([128, 512], f32, tag="osb")
                    nc.vector.tensor_add(o_sb, ps, bo_sb)
                    nc.sync.dma_start(out=out[b, lt * 128:(lt + 1) * 128, :], in_=o_sb)

            def pop_pending():
                ph, plc, pexp = pending.pop(0)
                ps_o = psum_ot.tile([128, 512], f32, tag="ot", name="ps_o")
                emit_ot(ph, plc, pexp, ps_o)
                emit_norm(ph, plc, ps_o)
                if (ph, plc) == (H - 1, 0):
                    emit_out_proj_half(0)  # all lc=0 heads normalized
                return plc

            pending = list(primed)
            for h in range(H):
                for lc in range(2):
                    if lc == 0 and h < len(primed):
                        continue  # scores already issued before the V proj
                    pending.append((h, lc, emit_chunk(h, lc)))
                    if len(pending) > 2:  # 2-deep stagger: OT runs two chunks behind
                        pop_pending()
            while pending:
                pop_pending()
            emit_out_proj_half(1)

    nc.compile()
    _CACHE["nc"] = nc
    return nc


def _in_maps(inputs):
    import ml_dtypes

    b16 = ml_dtypes.bfloat16
    e4 = ml_dtypes.float8_e4m3fn
    f = lambda a: np.ascontiguousarray(np.asarray(a, dtype=np.float32))
    queries, keys, values = f(inputs["queries"]), f(inputs["keys"]), f(inputs["values"])
    Wq, Wk, Wv, Wo = f(inputs["Wq"]), f(inputs["Wk"]), f(inputs["Wv"]), f(inputs["Wo"])
    bq, bk, bv, bo = f(inputs["bq"]), f(inputs["bk"]), f(inputs["bv"]), f(inputs["bo"])

    def w_dr(W):
        return np.ascontiguousarray(
            (W * WSCALE).reshape(2, 2, 128, -1).transpose(0, 2, 1, 3).astype(e4)
        )

    def x_dr(x):
        xt = x.transpose(0, 2, 1)
        bsz = xt.shape[0]
        return np.ascontiguousarray(
            xt.reshape(bsz, 2, 2, 128, -1).transpose(0, 1, 3, 2, 4).astype(e4)
        )

    shared = {
        "Wq": w_dr(Wq), "Wk": w_dr(Wk),
        "Wv": np.ascontiguousarray(Wv.astype(b16)), "Wo": Wo,
        "bq_col": np.ascontiguousarray(bq.reshape(4, 128).T) * WSCALE,
        "bk_col": np.ascontiguousarray(bk.reshape(4, 128).T) * WSCALE,
        "bv_row": bv, "bo_row": bo,
        "ones_in": np.ones((128, 128), np.float32),
    }
    maps = []
    for c in range(NCORES):
        sl = slice(BPC * c, BPC * (c + 1))
        maps.append({
            "qT": x_dr(queries[sl]),
            "kT": x_dr(keys[sl]),
            "vT": np.ascontiguousarray(values[sl].transpose(0, 2, 1).astype(b16)),
            **shared,
        })
    return maps


def kernel(**inputs) -> np.ndarray:
    global LAST_RESULTS
    from concourse import bass_utils

    nc = _build()
    maps = _in_maps(inputs)
    res = bass_utils.run_bass_kernel_spmd(nc, maps, core_ids=list(range(NCORES)))
    LAST_RESULTS = res
    return np.concatenate([res.results[c]["out"] for c in range(NCORES)], axis=0)

